# revision 14
# baseline (speedup 1.0000x reference)
"""4-layer LSTM decoder (nn_Decoder) on 8 Trainium2 NeuronCores.

Sharding: model-parallel over the gate/hidden dimension (each core owns 128
of the 1024 hidden units of every layer, i.e. 512 of the 4096 gate rows).
The sequential recurrence is scheduled as a wavefront over anti-diagonals
d = t + layer; each diagonal does all its gate GEMMs, the LSTM cells, then
AllGathers of the transposed hidden-state slices so every core has the full
h needed by the next diagonal.

v2 datapath:
- Gate GEMMs run in fp8 e4m3 with DoubleRow perf mode (K=256 per matmul,
  0.5 cycles/row): weights are pre-scaled x2048 and h is carried x16; the
  combined 1/32768 is folded into the sigmoid/tanh activation scale.
  Measured ~3.5x the bf16 matmul throughput; end-to-end error ~6e-3
  (fp32-reference), well inside the 2e-2 gate. PSUM accumulation, the cell
  state c, and all cell elementwise math stay fp32.
- The output projection path stays bf16 for accuracy: a bf16 copy of the
  layer-3 hidden state is smuggled inside the fp8 AllGather payload via a
  dtype bitcast, and L @ h3 runs as bf16 matmuls.
- hy is transposed for the gather with the XBAR transpose DMA
  (dma_start_transpose) instead of PE-transposes, freeing the tensor engine;
  the cell's last multiply emits bf16 directly.
- Per-diagonal communication: one merged AllGather (all 4 layers' h.T fp8
  slices + the bf16 h3), halving the number of cross-core sync points per
  diagonal vs the split variant; measured faster under fabric congestion
  and within noise otherwise. agmode="split2" restores two pipelined AGs
  (layer pairs (2,3) then (0,1)). The gather unpack is spread across both
  hardware DMA queues (SP + Activation) so the two layer-pair slots land in
  parallel, and the cell activations are emitted f-gate-first so the
  elementwise chain starts before the i/o sigmoids finish.
  (A 2-stream batch-split variant that hides AG latency behind the other
  stream's compute was measured slower: it doubles both the per-step weight
  streaming on the PE and the barrier count.)

The Runner keeps the jitted executable cached and supports device-resident
arguments (device_args/call_dev) so repeat calls skip the host upload.
"""
import sys
sys.path.insert(0, '/opt/trn_rl_repo')
import numpy as np

NLAYERS, NHID, NOUT, BSZ, STEPS = 4, 1024, 512, 64, 128
NC = 8           # cores
HS = NHID // NC  # 128 h-units per core
GS = 4 * HS      # 512 gate rows per core (i|f|o|c blocks of 128)
NOS = NOUT // NC  # 64 output cols per core
KCH = NHID // 128  # 8 contraction chunks (bf16 L path)
KP = 4             # fp8 DoubleRow pair-chunks (256 hidden each)
WSC, HSC = 2048.0, 16.0
ISC = 1.0 / (WSC * HSC)

_RUNNER_CACHE = {}


def _build(steps, out_steps=None, agmode="single"):
    import concourse.bass as bass
    import concourse.bacc as bacc
    import concourse.mybir as mybir
    from concourse.tile import TileContext

    f32 = mybir.dt.float32
    bf16 = mybir.dt.bfloat16
    fp8 = mybir.dt.float8e4
    AF = mybir.ActivationFunctionType
    DR = mybir.MatmulPerfMode.DoubleRow

    nc = bacc.Bacc(name="lstm_dec")
    # inputs (per-core slices prepared on host)
    h08_d = nc.dram_tensor("h08", [128, NLAYERS * 512], fp8, kind="ExternalInput")
    h03_d = nc.dram_tensor("h03", [128, NC * 64], bf16, kind="ExternalInput")
    c0_d = nc.dram_tensor("c0", [64, NLAYERS * HS], f32, kind="ExternalInput")
    u8_d = nc.dram_tensor("u8", [128, NLAYERS * KP * 1024], fp8, kind="ExternalInput")
    w8_d = nc.dram_tensor("w8", [128, (NLAYERS - 1) * KP * 1024], fp8, kind="ExternalInput")
    lt_d = nc.dram_tensor("lt", [128, KCH * NOS], bf16, kind="ExternalInput")
    out_d = nc.dram_tensor("out", [out_steps or steps, 64, NOS], f32, kind="ExternalOutput")

    ndiag = steps + 3  # diagonals 1..ndiag; cells (t,i): t=d-i in [1, steps-1]
    tmax = steps - 1

    def ht8_view(t):
        return t[:].rearrange("p (r l b) -> p r l b", r=NC, l=NLAYERS, b=64)

    with TileContext(nc) as tc:
        with (
            tc.tile_pool(name="wpool", bufs=1) as wpool,
            tc.tile_pool(name="ht", bufs=3) as htp,
            tc.tile_pool(name="ct", bufs=2) as ctp,
            tc.tile_pool(name="tmp", bufs=2) as tmp,
            tc.tile_pool(name="cellf", bufs=1) as cfp,
            tc.tile_pool(name="io", bufs=3) as iop,
            tc.tile_pool(name="ps", bufs=4, space="PSUM") as pp,
            tc.tile_pool(name="psl", bufs=2, space="PSUM") as ppl,
            tc.tile_pool(name="dram", bufs=4, space="DRAM") as dram,
        ):
            # --- weights arrive pre-cast/packed; DMA straight into SBUF ---
            u8 = wpool.tile([128, NLAYERS * KP * 1024], fp8, tag="u8")
            w8 = wpool.tile([128, (NLAYERS - 1) * KP * 1024], fp8, tag="w8")
            lt = wpool.tile([128, KCH * NOS], bf16, tag="lt")
            CH = 4096
            for dst, src_t, width in ((u8, u8_d, NLAYERS * KP * 1024),
                                      (w8, w8_d, (NLAYERS - 1) * KP * 1024),
                                      (lt, lt_d, KCH * NOS)):
                for off in range(0, width, CH):
                    w = min(CH, width - off)
                    nc.sync.dma_start(dst[:, off:off + w], src_t[:, off:off + w])
            h08sb = wpool.tile([128, NLAYERS * 512], fp8, tag="h08sb")
            nc.sync.dma_start(h08sb[:], h08_d[:])
            h03sb = wpool.tile([128, NC * 64], bf16, tag="h03sb")
            nc.sync.dma_start(h03sb[:], h03_d[:])
            ht8_init = htp.tile([128, NLAYERS * 512], fp8, tag="ht8")
            nc.vector.tensor_copy(ht8_init[:], h08sb[:])
            ht3_init = htp.tile([128, NC * 64], bf16, tag="ht3")
            nc.vector.tensor_copy(ht3_init[:], h03sb[:])
            ct_init = ctp.tile([64, NLAYERS * HS], f32, tag="ct")
            nc.sync.dma_start(ct_init[:], c0_d[:])

            ht8_read, ht3_read, ct_read = ht8_init, ht3_init, ct_init

            for d in range(1, ndiag + 1):
                ht3_lp = ht3_read
                cells = [(d - i, i) for i in range(NLAYERS) if 1 <= d - i <= tmax]
                cell_layers = {i for (_, i) in cells}
                full = len(cells) == NLAYERS
                do_pack = d <= ndiag - 1 and cells

                # --- gate GEMMs (fp8 DoubleRow, K=256/mm) ---
                # layer-grouped: group j emits U_j (into psum_j) and W_j (into
                # psum_{j+1}), sharing the ht8 layer-j stationary slices.
                # Deepest groups first so psum_3/psum_2 complete earliest.
                psums = {}
                for i in sorted(cell_layers, reverse=True):
                    ps_t = pp.tile([64, GS], f32, tag="gates")
                    psums[i] = ps_t[:]
                hv = ht8_view(ht8_read)
                # U0 last: psums 3,2,1 complete early so cells {3,2,1} overlap
                # the trailing U0 matmuls and only cell 0's short chain gates
                # the pack.
                for j, do_u, do_w in ((3, True, False), (2, True, True),
                                      (1, True, True), (0, False, True),
                                      (0, True, False)):
                    u_dst = j if (do_u and j in psums) else None
                    w_dst = j + 1 if (do_w and (j + 1) in psums) else None
                    for cp in range(KP):
                        lhs = hv[:, 2 * cp:2 * cp + 2, j, :]
                        if u_dst is not None:
                            nc.tensor.matmul(
                                psums[u_dst], lhs,
                                u8[:, (j * KP + cp) * 1024:(j * KP + cp + 1) * 1024]
                                .rearrange("p (k n) -> p k n", k=2),
                                start=(cp == 0),
                                stop=(cp == KP - 1 and j == 0),
                                perf_mode=DR)
                        if w_dst is not None:
                            nc.tensor.matmul(
                                psums[w_dst], lhs,
                                w8[:, (j * KP + cp) * 1024:(j * KP + cp + 1) * 1024]
                                .rearrange("p (k n) -> p k n", k=2),
                                start=False,
                                stop=(cp == KP - 1),
                                perf_mode=DR)

                # --- cells + pack ---
                ct_new = None
                if cells:
                    ct_new = ctp.tile([64, NLAYERS * HS], f32, tag="ct")
                ccis = []
                if agmode == "single" and do_pack:
                    cci_all = dram.tile([128, 256 + 128], fp8, tag="cc_in")
                if full:
                    assert agmode == "single"
                    for gi, grp in enumerate(((1, 2, 3), (0,))):
                        lo, nl = grp[0], len(grp)
                        S = cfp.tile([64, nl * 384], f32, tag=f"sifoG{gi}", name=f"sg{gi}")
                        T1 = cfp.tile([64, nl * HS], f32, tag=f"tccG{gi}", name=f"t1{gi}")
                        # f first (unblocks M1), then c-tanh + i (M2), o last (HY)
                        for i in reversed(grp):
                            nc.scalar.activation(S[:, (i - lo) * 384 + 128:(i - lo) * 384 + 256],
                                                 psums[i][:, 128:256], AF.Sigmoid, scale=ISC)
                        for i in reversed(grp):
                            nc.scalar.activation(T1[:, (i - lo) * HS:(i - lo + 1) * HS],
                                                 psums[i][:, 384:512], AF.Tanh, scale=ISC)
                            nc.scalar.activation(S[:, (i - lo) * 384:(i - lo) * 384 + 128],
                                                 psums[i][:, 0:128], AF.Sigmoid, scale=ISC)
                        for i in reversed(grp):
                            nc.scalar.activation(S[:, (i - lo) * 384 + 256:(i - lo + 1) * 384],
                                                 psums[i][:, 256:384], AF.Sigmoid, scale=ISC)
                        S3 = S[:].rearrange("b (l g) -> b l g", l=nl)
                        ctr2 = ct_read[:, lo * HS:(lo + nl) * HS].rearrange("b (l g) -> b l g", l=nl)
                        M1 = cfp.tile([64, nl * HS], f32, tag=f"m1G{gi}", name=f"m1{gi}")
                        nc.vector.tensor_mul(
                            M1[:].rearrange("b (l g) -> b l g", l=nl), S3[:, :, 128:256], ctr2)
                        M2 = cfp.tile([64, nl * HS], f32, tag=f"m2G{gi}", name=f"m2{gi}")
                        nc.vector.tensor_mul(
                            M2[:].rearrange("b (l g) -> b l g", l=nl), S3[:, :, 0:128],
                            T1[:].rearrange("b (l g) -> b l g", l=nl))
                        nc.vector.tensor_add(ct_new[:, lo * HS:(lo + nl) * HS], M1[:], M2[:])
                        TY = cfp.tile([64, nl * HS], f32, tag=f"tcyG{gi}", name=f"ty{gi}")
                        nc.scalar.activation(TY[:], ct_new[:, lo * HS:(lo + nl) * HS], AF.Tanh)
                        HYb = cfp.tile([64, nl * HS], bf16, tag=f"hyG{gi}", name=f"hy{gi}")
                        nc.vector.tensor_mul(
                            HYb[:].rearrange("b (l g) -> b l g", l=nl), S3[:, :, 256:384],
                            TY[:].rearrange("b (l g) -> b l g", l=nl))
                        if do_pack:
                            hyT = iop.tile([128, nl * 64], bf16, tag=f"hyTG{gi}", name=f"hyt{gi}")
                            for i in grp:
                                nc.scalar.dma_start_transpose(
                                    hyT[:, (i - lo) * 64:(i - lo + 1) * 64],
                                    HYb[:, (i - lo) * HS:(i - lo + 1) * HS])
                            hyT8 = iop.tile([128, nl * 64], fp8, tag=f"hyT8G{gi}", name=f"hyt8{gi}")
                            nc.vector.tensor_scalar_mul(hyT8[:], hyT[:], HSC)
                            if gi == 0:
                                # layers (2,3) -> cci cols 0:128, layer 1 -> 192:256,
                                # bf16 h3 -> 256:384
                                nc.scalar.dma_start(cci_all[:, 0:128], hyT8[:, 64:192])
                                nc.scalar.dma_start(cci_all[:, 192:256], hyT8[:, 0:64])
                                nc.scalar.dma_start(
                                    cci_all[:, 256:384].bitcast(bf16), hyT[:, 128:192])
                            else:
                                nc.scalar.dma_start(cci_all[:, 128:192], hyT8[:])
                else:
                    # per-cell chains (head/tail diagonals), deepest layer first
                    hyTa = iop.tile([128, 2 * 64], bf16, tag="hyT0")
                    hyTb = iop.tile([128, 2 * 64], bf16, tag="hyT1")
                    hyT01 = (hyTb, hyTa)  # index by i // 2
                    for (t, i) in reversed(cells):
                        sifo = tmp.tile([64, 384], f32, tag="sifo")
                        nc.scalar.activation(sifo[:], psums[i][:, :384], AF.Sigmoid, scale=ISC)
                        tcc = tmp.tile([64, HS], f32, tag="tcc")
                        nc.scalar.activation(tcc[:], psums[i][:, 384:512], AF.Tanh, scale=ISC)
                        m1 = tmp.tile([64, HS], f32, tag="m1")
                        nc.vector.tensor_mul(m1[:], sifo[:, 128:256], ct_read[:, i * HS:(i + 1) * HS])
                        m2 = tmp.tile([64, HS], f32, tag="m2")
                        nc.vector.tensor_mul(m2[:], sifo[:, 0:128], tcc[:])
                        nc.vector.tensor_add(ct_new[:, i * HS:(i + 1) * HS], m1[:], m2[:])
                        tcy = tmp.tile([64, HS], f32, tag="tcy")
                        nc.scalar.activation(tcy[:], ct_new[:, i * HS:(i + 1) * HS], AF.Tanh)
                        hyb = tmp.tile([64, HS], bf16, tag="hyb")
                        nc.vector.tensor_mul(hyb[:], sifo[:, 256:384], tcy[:])
                        nc.scalar.dma_start_transpose(
                            hyT01[i // 2][:, (i % 2) * 64:(i % 2 + 1) * 64], hyb[:])
                    # carry c for layers not yet started (early diagonals only)
                    if d <= NLAYERS:
                        for i in range(NLAYERS):
                            if i not in cell_layers and d <= i:
                                nc.vector.tensor_copy(
                                    ct_new[:, i * HS:(i + 1) * HS], ct_read[:, i * HS:(i + 1) * HS])
                    if do_pack:
                        for hk, pair in enumerate(((2, 3), (0, 1))):
                            lo = pair[0]
                            hyT = hyT01[1 - hk]
                            hyT8 = iop.tile([128, 2 * 64], fp8, tag=f"hyT8{hk}")
                            for i in pair:
                                sl = slice((i - lo) * 64, (i - lo + 1) * 64)
                                if i in cell_layers:
                                    nc.vector.tensor_scalar_mul(hyT8[:, sl], hyT[:, sl], HSC)
                                else:
                                    nc.vector.tensor_scalar_mul(
                                        hyT8[:, sl], h08sb[:, 0:64], 0.0)
                            if agmode == "single":
                                nc.scalar.dma_start(cci_all[:, hk * 128:(hk + 1) * 128], hyT8[:])
                                if hk == 0 and 3 in cell_layers:
                                    nc.scalar.dma_start(
                                        cci_all[:, 256:384].bitcast(bf16), hyT[:, 64:128])
                                elif hk == 0:
                                    z3 = iop.tile([128, 128], fp8, tag="z3")
                                    nc.vector.tensor_scalar_mul(z3[:], h08sb[:, 0:128], 0.0)
                                    nc.scalar.dma_start(cci_all[:, 256:384], z3[:])
                            else:
                                w_cols = 256 if hk == 0 else 128
                                cci = dram.tile([128, w_cols], fp8, tag=f"cc_in{hk}",
                                                name=f"cci{hk}")
                                nc.scalar.dma_start(cci[:, 0:128], hyT8[:])
                                if hk == 0 and 3 in cell_layers:
                                    nc.scalar.dma_start(
                                        cci[:, 128:256].bitcast(bf16), hyT[:, 64:128])
                                elif hk == 0:
                                    z3 = iop.tile([128, 128], fp8, tag="z3")
                                    nc.vector.tensor_scalar_mul(z3[:], h08sb[:, 0:128], 0.0)
                                    nc.scalar.dma_start(cci[:, 128:256], z3[:])
                                ccis.append(cci)

                # --- AllGather + unpack ---
                if do_pack:
                    ht8_new = htp.tile([128, NLAYERS * 512], fp8, tag="ht8")
                    has3 = 3 in cell_layers
                    if has3:
                        ht3_new = htp.tile([128, NC * 64], bf16, tag="ht3")
                    if agmode == "single":
                        cco = dram.tile([NC * 128, 256 + 128], fp8, tag="cc_out")
                        nc.gpsimd.collective_compute(
                            "AllGather", mybir.AluOpType.bypass,
                            replica_groups=[list(range(NC))],
                            ins=[cci_all[:].opt()], outs=[cco[:].opt()])
                        # per-layer unpack in consumption order (l3 first),
                        # spread across both DMA queues
                        for l, col, eng in ((3, 1, nc.sync), (1, 3, nc.scalar),
                                            (2, 0, nc.sync), (0, 2, nc.scalar)):
                            eng.dma_start(
                                ht8_view(ht8_new)[:, :, l, :],
                                cco[:, col * 64:(col + 1) * 64]
                                .rearrange("(r p) b -> p r b", p=128))
                        if has3:
                            nc.sync.dma_start(
                                ht3_new[:].rearrange("p (r b) -> p r b", r=NC),
                                cco[:, 256:384].bitcast(bf16)
                                .rearrange("(r p) b -> p r b", p=128))
                    else:
                        for hk, lo in ((0, 2), (1, 0)):
                            cci = ccis[hk]
                            w_cols = 256 if hk == 0 else 128
                            cco = dram.tile([NC * 128, w_cols], fp8, tag=f"cc_out{hk}",
                                            name=f"cco{hk}")
                            nc.gpsimd.collective_compute(
                                "AllGather", mybir.AluOpType.bypass,
                                replica_groups=[list(range(NC))],
                                ins=[cci[:].opt()], outs=[cco[:].opt()])
                            nc.sync.dma_start(
                                ht8_view(ht8_new)[:, :, lo:lo + 2, :],
                                cco[:, 0:128]
                                .rearrange("(r p) (l b) -> p r l b", p=128, l=2))
                            if hk == 0 and has3:
                                nc.sync.dma_start(
                                    ht3_new[:].rearrange("p (r b) -> p r b", r=NC),
                                    cco[:, 128:256].bitcast(bf16)
                                    .rearrange("(r p) b -> p r b", p=128))
                    # layers not yet started: fill slots locally from init
                    for i in range(NLAYERS):
                        if d - i < 1:
                            nc.vector.tensor_copy(
                                ht8_view(ht8_new)[:, :, i, :],
                                ht8_view(h08sb)[:, :, i, :])
                    ht8_read = ht8_new
                    if has3:
                        ht3_read = ht3_new

                # --- L projection for t_L (bf16; needs gathered h3 of t_L) ---
                t_L = 0 if d == 1 else (d - 4 if 5 <= d <= ndiag else None)
                if t_L is not None:
                    psl = ppl.tile([64, NOS], f32, tag="lproj")
                    for ch in range(KCH):
                        nc.tensor.matmul(
                            psl[:], ht3_lp[:, ch * 64:(ch + 1) * 64],
                            lt[:, ch * NOS:(ch + 1) * NOS],
                            start=(ch == 0), stop=(ch == KCH - 1))
                    so = iop.tile([64, NOS], f32, tag="so")
                    nc.vector.tensor_copy(so[:], psl[:])
                    nc.sync.dma_start(out_d[t_L, :, :], so[:])

                if cells:
                    ct_read = ct_new
    nc.finalize()
    return nc


def _prep_inputs(hx, cx, W, U, L):
    hx = np.asarray(hx, np.float32)
    cx = np.asarray(cx, np.float32)
    W = np.asarray(W, np.float32)
    U = np.asarray(U, np.float32)
    L = np.asarray(L, np.float32)
    import ml_dtypes
    e4, bf = ml_dtypes.float8_e4m3, ml_dtypes.bfloat16

    def q8(x, s):
        return np.clip(x * s, -240.0, 240.0).astype(e4)

    # gathered-h layouts (rank-major): h08[p, r*256+l*64+b] = 16*hx[l, b, r*128+p]
    h0 = hx.transpose(0, 2, 1).reshape(NLAYERS, NC, 128, 64)  # [l, r, p, b]
    h0 = h0.transpose(2, 1, 0, 3).reshape(128, NLAYERS * 512)
    h08 = q8(np.ascontiguousarray(h0), HSC)
    h3 = hx[3].T.reshape(NC, 128, 64).transpose(1, 0, 2).reshape(128, NC * 64)
    h03 = np.ascontiguousarray(h3).astype(bf)

    ins = []
    for k in range(NC):
        # local gate rows in [i|f|o|c] order: global U/W rows
        rows = np.concatenate([
            np.arange(0 * NHID + k * HS, 0 * NHID + (k + 1) * HS),   # i
            np.arange(1 * NHID + k * HS, 1 * NHID + (k + 1) * HS),   # f
            np.arange(3 * NHID + k * HS, 3 * NHID + (k + 1) * HS),   # o
            np.arange(2 * NHID + k * HS, 2 * NHID + (k + 1) * HS),   # c
        ])
        # u8[p, ((l*KP+cp)*2+k2)*512 + n] = 2048*U[l, rows[n], cp*256+k2*128+p]
        Usl = U[:, rows, :]                       # [l, 512, 1024]
        ut = Usl.reshape(NLAYERS, GS, KP, 2, 128).transpose(4, 0, 2, 3, 1)
        u8 = q8(np.ascontiguousarray(ut.reshape(128, NLAYERS * KP * 1024)), WSC)
        Wsl = W[:, rows, :]                       # [3, 512, 1024]
        wt = Wsl.reshape(NLAYERS - 1, GS, KP, 2, 128).transpose(4, 0, 2, 3, 1)
        w8 = q8(np.ascontiguousarray(wt.reshape(128, (NLAYERS - 1) * KP * 1024)), WSC)
        # lt[p, ch*NOS + j] = L[k*NOS + j, ch*128 + p]
        Lsl = L[k * NOS:(k + 1) * NOS, :]         # [64, 1024]
        ltk = Lsl.reshape(NOS, KCH, 128).transpose(2, 1, 0)
        ltk = np.ascontiguousarray(ltk.reshape(128, KCH * NOS)).astype(bf)
        # c0[b, l*HS + j] = cx[l, b, k*HS + j]
        c0 = cx[:, :, k * HS:(k + 1) * HS].transpose(1, 0, 2)
        c0 = np.ascontiguousarray(c0.reshape(64, NLAYERS * HS))
        ins.append({"h08": h08, "h03": h03, "c0": c0, "u8": u8, "w8": w8, "lt": ltk})
    return ins


class _Runner:
    def __init__(self, nc, n_cores=NC, donate=False):
        import jax
        from jax.sharding import Mesh, PartitionSpec
        from jax.experimental.shard_map import shard_map
        from concourse import bass2jax, mybir
        bass2jax.install_neuronx_cc_hook()
        self.n_cores = n_cores
        partition_name = nc.partition_id_tensor.name if nc.partition_id_tensor else None
        in_names, out_names, out_avals, zero_outs = [], [], [], []
        for alloc in nc.m.functions[0].allocations:
            if not isinstance(alloc, mybir.MemoryLocationSet):
                continue
            name = alloc.memorylocations[0].name
            if alloc.kind == "ExternalInput":
                if name != partition_name:
                    in_names.append(name)
            elif alloc.kind == "ExternalOutput":
                out_names.append(name)
                shape = tuple(alloc.tensor_shape)
                dtype = mybir.dt.np(alloc.dtype)
                out_avals.append(jax.core.ShapedArray(shape, dtype))
                zero_outs.append(np.zeros(shape, dtype))
        self.in_names, self.out_names = in_names, out_names
        self.out_avals, self.zero_outs = out_avals, zero_outs
        n_params = len(in_names)
        self.n_params = n_params
        all_in_names = in_names + out_names
        if partition_name is not None:
            all_in_names.append(partition_name)
        donate_idx = tuple(range(n_params, n_params + len(out_avals))) if donate else ()

        def _body(*args):
            operands = list(args)
            if partition_name is not None:
                operands.append(bass2jax.partition_id_tensor())
            outs = bass2jax._bass_exec_p.bind(
                *operands, out_avals=tuple(out_avals), in_names=tuple(all_in_names),
                out_names=tuple(out_names), lowering_input_output_aliases=(),
                sim_require_finite=False, sim_require_nnan=False, nc=nc)
            return tuple(outs)

        devices = jax.devices()[:n_cores]
        mesh = Mesh(np.asarray(devices), ("core",))
        in_specs = (PartitionSpec("core"),) * (n_params + len(out_avals))
        out_specs = (PartitionSpec("core"),) * len(out_names)
        self.fn = jax.jit(
            shard_map(_body, mesh=mesh, in_specs=in_specs, out_specs=out_specs,
                      check_rep=False),
            donate_argnums=donate_idx, keep_unused=True)
        self._jax = jax

    def device_args(self, in_maps):
        """Upload per-core inputs + zero output buffers once; reusable across calls."""
        import jax
        from jax.sharding import Mesh, NamedSharding, PartitionSpec
        per_core = [[np.asarray(m[n]) for n in self.in_names] for m in in_maps]
        concat_in = [np.concatenate([per_core[c][i] for c in range(self.n_cores)], axis=0)
                     for i in range(self.n_params)]
        concat_zeros = [np.zeros((self.n_cores * z.shape[0], *z.shape[1:]), z.dtype)
                        for z in self.zero_outs]
        mesh = Mesh(np.asarray(jax.devices()[:self.n_cores]), ("core",))
        sh = NamedSharding(mesh, PartitionSpec("core"))
        return [jax.device_put(a, sh) for a in concat_in + concat_zeros]

    def call_dev(self, dev_args):
        out_arrs = self.fn(*dev_args)
        self._jax.block_until_ready(out_arrs)
        return out_arrs

    def __call__(self, in_maps):
        out_arrs = self.call_dev(self.device_args(in_maps))
        return [
            {n: np.asarray(out_arrs[i]).reshape(self.n_cores, *self.out_avals[i].shape)[c]
             for i, n in enumerate(self.out_names)}
            for c in range(self.n_cores)
        ]


def _get_runner(steps):
    if steps not in _RUNNER_CACHE:
        nc = _build(steps)
        _RUNNER_CACHE[steps] = _Runner(nc)
    return _RUNNER_CACHE[steps]


def kernel(hx, cx, W, U, L, steps):
    steps = int(steps)
    ins = _prep_inputs(hx, cx, W, U, L)
    runner = _get_runner(steps)
    res = runner(ins)
    out = np.concatenate([res[k]["out"] for k in range(NC)], axis=2)  # [steps, 64, 512]
    return out.astype(np.float32)


# revision 16
# speedup vs baseline: 1.1851x; 1.1851x over previous
"""4-layer LSTM decoder (nn_Decoder) on 8 Trainium2 NeuronCores.

Sharding: model-parallel over the gate/hidden dimension (each core owns 128
of the 1024 hidden units of every layer, i.e. 512 of the 4096 gate rows).
The sequential recurrence is scheduled as a wavefront over anti-diagonals
d = t + layer; each diagonal does all its gate GEMMs, the LSTM cells, then
AllGathers of the transposed hidden-state slices so every core has the full
h needed by the next diagonal.

v2 datapath:
- Gate GEMMs run in fp8 e4m3 with DoubleRow perf mode (K=256 per matmul,
  0.5 cycles/row): weights are pre-scaled x2048 and h is carried x16; the
  combined 1/32768 is folded into the sigmoid/tanh activation scale.
  Measured ~3.5x the bf16 matmul throughput; end-to-end error ~6e-3
  (fp32-reference), well inside the 2e-2 gate. PSUM accumulation, the cell
  state c, and all cell elementwise math stay fp32.
- The output projection path stays bf16 for accuracy: the unpack's layer-3
  transpose output is kept in bf16 and feeds L @ h3 directly.
- Communication: cells DMA their bf16 hy straight to the gather input in
  [batch, hidden] layout (no pack-side transpose or convert), one merged
  AllGather per diagonal, then per-layer XBAR transpose DMAs read the
  gathered result straight out of DRAM (layer 3 first, spread across both
  DMA queues) followed by small DVE converts to the fp8 stationary. This is
  one serial DMA hop fewer per diagonal than transpose-at-pack (each DMA
  carries ~1.3-1.7us init latency), at the cost of a 0.36us convert.
- The matmuls emit U0 last so psums 3,2,1 complete early: cells {3,2,1}
  run as one fused block overlapping the trailing U0 matmuls, and only
  cell 0's short single-cell chain (f-gate-first activations) gates the
  pack+AllGather.
  (Measured dead ends: two pipelined AllGathers per diagonal, a 2-stream
  batch split hiding AG latency behind the other stream's compute, bf16
  column-tiled matmuls, and weights-stationary M=128 matmuls.)

The Runner keeps the jitted executable cached and supports device-resident
arguments (device_args/call_dev) so repeat calls skip the host upload.
"""
import sys
sys.path.insert(0, '/opt/trn_rl_repo')
import numpy as np

NLAYERS, NHID, NOUT, BSZ, STEPS = 4, 1024, 512, 64, 128
NC = 8           # cores
HS = NHID // NC  # 128 h-units per core
GS = 4 * HS      # 512 gate rows per core (i|f|o|c blocks of 128)
NOS = NOUT // NC  # 64 output cols per core
KCH = NHID // 128  # 8 contraction chunks (bf16 L path)
KP = 4             # fp8 DoubleRow pair-chunks (256 hidden each)
WSC, HSC = 2048.0, 16.0
ISC = 1.0 / (WSC * HSC)

_RUNNER_CACHE = {}


def _build(steps, out_steps=None, agmode="single"):
    import concourse.bass as bass
    import concourse.bacc as bacc
    import concourse.mybir as mybir
    from concourse.tile import TileContext

    f32 = mybir.dt.float32
    bf16 = mybir.dt.bfloat16
    fp8 = mybir.dt.float8e4
    AF = mybir.ActivationFunctionType
    DR = mybir.MatmulPerfMode.DoubleRow

    nc = bacc.Bacc(name="lstm_dec")
    # inputs (per-core slices prepared on host)
    h08_d = nc.dram_tensor("h08", [128, NLAYERS * 512], fp8, kind="ExternalInput")
    h03_d = nc.dram_tensor("h03", [128, NC * 64], bf16, kind="ExternalInput")
    # h08 layout is layer-major: h08[p, l*512 + r*64 + b]
    c0_d = nc.dram_tensor("c0", [64, NLAYERS * HS], f32, kind="ExternalInput")
    u8_d = nc.dram_tensor("u8", [128, NLAYERS * KP * 1024], fp8, kind="ExternalInput")
    w8_d = nc.dram_tensor("w8", [128, (NLAYERS - 1) * KP * 1024], fp8, kind="ExternalInput")
    lt_d = nc.dram_tensor("lt", [128, KCH * NOS], bf16, kind="ExternalInput")
    out_d = nc.dram_tensor("out", [out_steps or steps, 64, NOS], f32, kind="ExternalOutput")

    ndiag = steps + 3  # diagonals 1..ndiag; cells (t,i): t=d-i in [1, steps-1]
    tmax = steps - 1

    def ht8_view(t):
        # layer-major: [p, (l r b)]
        return t[:].rearrange("p (l r b) -> p l r b", l=NLAYERS, r=NC, b=64)

    with TileContext(nc) as tc:
        with (
            tc.tile_pool(name="wpool", bufs=1) as wpool,
            tc.tile_pool(name="ht", bufs=3) as htp,
            tc.tile_pool(name="ct", bufs=2) as ctp,
            tc.tile_pool(name="tmp", bufs=2) as tmp,
            tc.tile_pool(name="cellf", bufs=1) as cfp,
            tc.tile_pool(name="io", bufs=3) as iop,
            tc.tile_pool(name="ps", bufs=4, space="PSUM") as pp,
            tc.tile_pool(name="psl", bufs=2, space="PSUM") as ppl,
            tc.tile_pool(name="dram", bufs=4, space="DRAM") as dram,
        ):
            # --- weights arrive pre-cast/packed; DMA straight into SBUF ---
            u8 = wpool.tile([128, NLAYERS * KP * 1024], fp8, tag="u8")
            w8 = wpool.tile([128, (NLAYERS - 1) * KP * 1024], fp8, tag="w8")
            lt = wpool.tile([128, KCH * NOS], bf16, tag="lt")
            CH = 4096
            for dst, src_t, width in ((u8, u8_d, NLAYERS * KP * 1024),
                                      (w8, w8_d, (NLAYERS - 1) * KP * 1024),
                                      (lt, lt_d, KCH * NOS)):
                for off in range(0, width, CH):
                    w = min(CH, width - off)
                    nc.sync.dma_start(dst[:, off:off + w], src_t[:, off:off + w])
            h08sb = wpool.tile([128, NLAYERS * 512], fp8, tag="h08sb")
            nc.sync.dma_start(h08sb[:], h08_d[:])
            h03sb = wpool.tile([128, NC * 64], bf16, tag="h03sb")
            nc.sync.dma_start(h03sb[:], h03_d[:])
            ht8_init = htp.tile([128, NLAYERS * 512], fp8, tag="ht8")
            nc.vector.tensor_copy(ht8_init[:], h08sb[:])
            ht3_init = htp.tile([128, NC * 64], bf16, tag="ht3")
            nc.vector.tensor_copy(ht3_init[:], h03sb[:])
            ct_init = ctp.tile([64, NLAYERS * HS], f32, tag="ct")
            nc.sync.dma_start(ct_init[:], c0_d[:])
            zb0 = wpool.tile([64, HS], bf16, tag="zb0")
            nc.vector.tensor_scalar_mul(zb0[:], ct_init[:, 0:HS], 0.0)

            ht8_read, ht3_read, ct_read = ht8_init, ht3_init, ct_init

            for d in range(1, ndiag + 1):
                ht3_lp = ht3_read
                cells = [(d - i, i) for i in range(NLAYERS) if 1 <= d - i <= tmax]
                cell_layers = {i for (_, i) in cells}
                full = len(cells) == NLAYERS
                do_pack = d <= ndiag - 1 and cells

                # --- gate GEMMs (fp8 DoubleRow, K=256/mm) ---
                # layer-grouped: group j emits U_j (into psum_j) and W_j (into
                # psum_{j+1}), sharing the ht8 layer-j stationary slices.
                # Deepest groups first so psum_3/psum_2 complete earliest.
                psums = {}
                for i in sorted(cell_layers, reverse=True):
                    ps_t = pp.tile([64, GS], f32, tag="gates")
                    psums[i] = ps_t[:]
                hv = ht8_view(ht8_read)
                # U0 last: psums 3,2,1 complete early so cells {3,2,1} overlap
                # the trailing U0 matmuls and only cell 0's short chain gates
                # the pack.
                for j, do_u, do_w in ((3, True, False), (2, True, True),
                                      (1, True, True), (0, False, True),
                                      (0, True, False)):
                    u_dst = j if (do_u and j in psums) else None
                    w_dst = j + 1 if (do_w and (j + 1) in psums) else None
                    for cp in range(KP):
                        lhs = hv[:, j, 2 * cp:2 * cp + 2, :]
                        if u_dst is not None:
                            nc.tensor.matmul(
                                psums[u_dst], lhs,
                                u8[:, (j * KP + cp) * 1024:(j * KP + cp + 1) * 1024]
                                .rearrange("p (k n) -> p k n", k=2),
                                start=(cp == 0),
                                stop=(cp == KP - 1 and j == 0),
                                perf_mode=DR)
                        if w_dst is not None:
                            nc.tensor.matmul(
                                psums[w_dst], lhs,
                                w8[:, (j * KP + cp) * 1024:(j * KP + cp + 1) * 1024]
                                .rearrange("p (k n) -> p k n", k=2),
                                start=False,
                                stop=(cp == KP - 1),
                                perf_mode=DR)

                # --- cells + pack ---
                ct_new = None
                if cells:
                    ct_new = ctp.tile([64, NLAYERS * HS], f32, tag="ct")
                ccis = []
                if agmode == "single" and do_pack:
                    cci_all = dram.tile([64, NLAYERS * HS], bf16, tag="cc_in")
                if full:
                    assert agmode == "single"
                    for gi, grp in enumerate(((1, 2, 3), (0,))):
                        lo, nl = grp[0], len(grp)
                        S = cfp.tile([64, nl * 384], f32, tag=f"sifoG{gi}", name=f"sg{gi}")
                        T1 = cfp.tile([64, nl * HS], f32, tag=f"tccG{gi}", name=f"t1{gi}")
                        # f first (unblocks M1), then c-tanh + i (M2), o last (HY)
                        for i in reversed(grp):
                            nc.scalar.activation(S[:, (i - lo) * 384 + 128:(i - lo) * 384 + 256],
                                                 psums[i][:, 128:256], AF.Sigmoid, scale=ISC)
                        for i in reversed(grp):
                            nc.scalar.activation(T1[:, (i - lo) * HS:(i - lo + 1) * HS],
                                                 psums[i][:, 384:512], AF.Tanh, scale=ISC)
                            nc.scalar.activation(S[:, (i - lo) * 384:(i - lo) * 384 + 128],
                                                 psums[i][:, 0:128], AF.Sigmoid, scale=ISC)
                        for i in reversed(grp):
                            nc.scalar.activation(S[:, (i - lo) * 384 + 256:(i - lo + 1) * 384],
                                                 psums[i][:, 256:384], AF.Sigmoid, scale=ISC)
                        S3 = S[:].rearrange("b (l g) -> b l g", l=nl)
                        ctr2 = ct_read[:, lo * HS:(lo + nl) * HS].rearrange("b (l g) -> b l g", l=nl)
                        M1 = cfp.tile([64, nl * HS], f32, tag=f"m1G{gi}", name=f"m1{gi}")
                        nc.vector.tensor_mul(
                            M1[:].rearrange("b (l g) -> b l g", l=nl), S3[:, :, 128:256], ctr2)
                        M2 = cfp.tile([64, nl * HS], f32, tag=f"m2G{gi}", name=f"m2{gi}")
                        nc.vector.tensor_mul(
                            M2[:].rearrange("b (l g) -> b l g", l=nl), S3[:, :, 0:128],
                            T1[:].rearrange("b (l g) -> b l g", l=nl))
                        nc.vector.tensor_add(ct_new[:, lo * HS:(lo + nl) * HS], M1[:], M2[:])
                        TY = cfp.tile([64, nl * HS], f32, tag=f"tcyG{gi}", name=f"ty{gi}")
                        nc.scalar.activation(TY[:], ct_new[:, lo * HS:(lo + nl) * HS], AF.Tanh)
                        HYb = cfp.tile([64, nl * HS], bf16, tag=f"hyG{gi}", name=f"hy{gi}")
                        nc.vector.tensor_mul(
                            HYb[:].rearrange("b (l g) -> b l g", l=nl), S3[:, :, 256:384],
                            TY[:].rearrange("b (l g) -> b l g", l=nl))
                        if do_pack:
                            # untransposed bf16 pack: cci[b, l*128 + h_local]
                            nc.scalar.dma_start(
                                cci_all[:, lo * HS:(lo + nl) * HS], HYb[:])
                else:
                    # per-cell chains (head/tail diagonals), deepest layer first
                    for (t, i) in reversed(cells):
                        sifo = tmp.tile([64, 384], f32, tag="sifo")
                        nc.scalar.activation(sifo[:], psums[i][:, :384], AF.Sigmoid, scale=ISC)
                        tcc = tmp.tile([64, HS], f32, tag="tcc")
                        nc.scalar.activation(tcc[:], psums[i][:, 384:512], AF.Tanh, scale=ISC)
                        m1 = tmp.tile([64, HS], f32, tag="m1")
                        nc.vector.tensor_mul(m1[:], sifo[:, 128:256], ct_read[:, i * HS:(i + 1) * HS])
                        m2 = tmp.tile([64, HS], f32, tag="m2")
                        nc.vector.tensor_mul(m2[:], sifo[:, 0:128], tcc[:])
                        nc.vector.tensor_add(ct_new[:, i * HS:(i + 1) * HS], m1[:], m2[:])
                        tcy = tmp.tile([64, HS], f32, tag="tcy")
                        nc.scalar.activation(tcy[:], ct_new[:, i * HS:(i + 1) * HS], AF.Tanh)
                        hyb = tmp.tile([64, HS], bf16, tag="hyb")
                        nc.vector.tensor_mul(hyb[:], sifo[:, 256:384], tcy[:])
                        if do_pack:
                            nc.scalar.dma_start(cci_all[:, i * HS:(i + 1) * HS], hyb[:])
                    # carry c for layers not yet started (early diagonals only)
                    if d <= NLAYERS:
                        for i in range(NLAYERS):
                            if i not in cell_layers and d <= i:
                                nc.vector.tensor_copy(
                                    ct_new[:, i * HS:(i + 1) * HS], ct_read[:, i * HS:(i + 1) * HS])
                    if do_pack:
                        for i in range(NLAYERS):
                            if i not in cell_layers:
                                nc.scalar.dma_start(cci_all[:, i * HS:(i + 1) * HS], zb0[:])

                # --- AllGather + unpack (transpose-at-unpack) ---
                if do_pack:
                    assert agmode == "single"
                    ht8_new = htp.tile([128, NLAYERS * 512], fp8, tag="ht8")
                    has3 = 3 in cell_layers
                    if has3:
                        ht3_new = htp.tile([128, NC * 64], bf16, tag="ht3")
                    cco = dram.tile([NC * 64, NLAYERS * HS], bf16, tag="cc_out")
                    nc.gpsimd.collective_compute(
                        "AllGather", mybir.AluOpType.bypass,
                        replica_groups=[list(range(NC))],
                        ins=[cci_all[:].opt()], outs=[cco[:].opt()])
                    # per-layer XBAR transpose DMA straight from the gathered
                    # DRAM ([r*64+b, l*128+h] -> [h, r*64+b]) then a small DVE
                    # convert to the fp8 stationary: one serial DMA hop fewer
                    # than transpose-at-pack. Layer 3 lands first (next
                    # diagonal's U3 + the L projection consume it).
                    guards = {}
                    for l, eng in ((3, nc.sync), (1, nc.scalar),
                                   (2, nc.sync), (0, nc.scalar)):
                        if l == 3 and has3:
                            tb = ht3_new
                        else:
                            tb = iop.tile([128, NC * 64], bf16, tag=f"tb{l}",
                                          name=f"tb{l}")
                        # guard: a plain DMA read of the gathered tensor on the
                        # same queue carries the HW-proven wait-on-collective
                        # edge; the in-order queue then orders the transpose
                        # behind it (the transpose's own wait wiring races the
                        # collective on HW).
                        g = iop.tile([64, 64], bf16, tag=f"gd{l}", name=f"gd{l}")
                        eng.dma_start(g[:], cco[0:64, l * HS:l * HS + 64])
                        guards[l] = g
                        eng.dma_start_transpose(tb[:], cco[:, l * HS:(l + 1) * HS])
                        nc.vector.tensor_scalar_mul(
                            ht8_new[:, l * 512:(l + 1) * 512], tb[:], HSC)
                    # layers not yet started: fill slots locally from init
                    for i in range(NLAYERS):
                        if d - i < 1:
                            nc.vector.tensor_copy(
                                ht8_view(ht8_new)[:, i, :, :],
                                ht8_view(h08sb)[:, i, :, :])
                    ht8_read = ht8_new
                    if has3:
                        ht3_read = ht3_new

                # --- L projection for t_L (bf16; needs gathered h3 of t_L) ---
                t_L = 0 if d == 1 else (d - 4 if 5 <= d <= ndiag else None)
                if t_L is not None:
                    psl = ppl.tile([64, NOS], f32, tag="lproj")
                    for ch in range(KCH):
                        nc.tensor.matmul(
                            psl[:], ht3_lp[:, ch * 64:(ch + 1) * 64],
                            lt[:, ch * NOS:(ch + 1) * NOS],
                            start=(ch == 0), stop=(ch == KCH - 1))
                    so = iop.tile([64, NOS], f32, tag="so")
                    nc.vector.tensor_copy(so[:], psl[:])
                    nc.sync.dma_start(out_d[t_L, :, :], so[:])

                if cells:
                    ct_read = ct_new
    nc.finalize()
    return nc


def _prep_inputs(hx, cx, W, U, L):
    hx = np.asarray(hx, np.float32)
    cx = np.asarray(cx, np.float32)
    W = np.asarray(W, np.float32)
    U = np.asarray(U, np.float32)
    L = np.asarray(L, np.float32)
    import ml_dtypes
    e4, bf = ml_dtypes.float8_e4m3, ml_dtypes.bfloat16

    def q8(x, s):
        return np.clip(x * s, -240.0, 240.0).astype(e4)

    # gathered-h layouts (rank-major): h08[p, r*256+l*64+b] = 16*hx[l, b, r*128+p]
    h0 = hx.transpose(0, 2, 1).reshape(NLAYERS, NC, 128, 64)  # [l, r, p, b]
    h0 = h0.transpose(2, 0, 1, 3).reshape(128, NLAYERS * 512)  # [p, l, r, b]
    h08 = q8(np.ascontiguousarray(h0), HSC)
    h3 = hx[3].T.reshape(NC, 128, 64).transpose(1, 0, 2).reshape(128, NC * 64)
    h03 = np.ascontiguousarray(h3).astype(bf)

    ins = []
    for k in range(NC):
        # local gate rows in [i|f|o|c] order: global U/W rows
        rows = np.concatenate([
            np.arange(0 * NHID + k * HS, 0 * NHID + (k + 1) * HS),   # i
            np.arange(1 * NHID + k * HS, 1 * NHID + (k + 1) * HS),   # f
            np.arange(3 * NHID + k * HS, 3 * NHID + (k + 1) * HS),   # o
            np.arange(2 * NHID + k * HS, 2 * NHID + (k + 1) * HS),   # c
        ])
        # u8[p, ((l*KP+cp)*2+k2)*512 + n] = 2048*U[l, rows[n], cp*256+k2*128+p]
        Usl = U[:, rows, :]                       # [l, 512, 1024]
        ut = Usl.reshape(NLAYERS, GS, KP, 2, 128).transpose(4, 0, 2, 3, 1)
        u8 = q8(np.ascontiguousarray(ut.reshape(128, NLAYERS * KP * 1024)), WSC)
        Wsl = W[:, rows, :]                       # [3, 512, 1024]
        wt = Wsl.reshape(NLAYERS - 1, GS, KP, 2, 128).transpose(4, 0, 2, 3, 1)
        w8 = q8(np.ascontiguousarray(wt.reshape(128, (NLAYERS - 1) * KP * 1024)), WSC)
        # lt[p, ch*NOS + j] = L[k*NOS + j, ch*128 + p]
        Lsl = L[k * NOS:(k + 1) * NOS, :]         # [64, 1024]
        ltk = Lsl.reshape(NOS, KCH, 128).transpose(2, 1, 0)
        ltk = np.ascontiguousarray(ltk.reshape(128, KCH * NOS)).astype(bf)
        # c0[b, l*HS + j] = cx[l, b, k*HS + j]
        c0 = cx[:, :, k * HS:(k + 1) * HS].transpose(1, 0, 2)
        c0 = np.ascontiguousarray(c0.reshape(64, NLAYERS * HS))
        ins.append({"h08": h08, "h03": h03, "c0": c0, "u8": u8, "w8": w8, "lt": ltk})
    return ins


class _Runner:
    def __init__(self, nc, n_cores=NC, donate=False):
        import jax
        from jax.sharding import Mesh, PartitionSpec
        from jax.experimental.shard_map import shard_map
        from concourse import bass2jax, mybir
        bass2jax.install_neuronx_cc_hook()
        self.n_cores = n_cores
        partition_name = nc.partition_id_tensor.name if nc.partition_id_tensor else None
        in_names, out_names, out_avals, zero_outs = [], [], [], []
        for alloc in nc.m.functions[0].allocations:
            if not isinstance(alloc, mybir.MemoryLocationSet):
                continue
            name = alloc.memorylocations[0].name
            if alloc.kind == "ExternalInput":
                if name != partition_name:
                    in_names.append(name)
            elif alloc.kind == "ExternalOutput":
                out_names.append(name)
                shape = tuple(alloc.tensor_shape)
                dtype = mybir.dt.np(alloc.dtype)
                out_avals.append(jax.core.ShapedArray(shape, dtype))
                zero_outs.append(np.zeros(shape, dtype))
        self.in_names, self.out_names = in_names, out_names
        self.out_avals, self.zero_outs = out_avals, zero_outs
        n_params = len(in_names)
        self.n_params = n_params
        all_in_names = in_names + out_names
        if partition_name is not None:
            all_in_names.append(partition_name)
        donate_idx = tuple(range(n_params, n_params + len(out_avals))) if donate else ()

        def _body(*args):
            operands = list(args)
            if partition_name is not None:
                operands.append(bass2jax.partition_id_tensor())
            outs = bass2jax._bass_exec_p.bind(
                *operands, out_avals=tuple(out_avals), in_names=tuple(all_in_names),
                out_names=tuple(out_names), lowering_input_output_aliases=(),
                sim_require_finite=False, sim_require_nnan=False, nc=nc)
            return tuple(outs)

        devices = jax.devices()[:n_cores]
        mesh = Mesh(np.asarray(devices), ("core",))
        in_specs = (PartitionSpec("core"),) * (n_params + len(out_avals))
        out_specs = (PartitionSpec("core"),) * len(out_names)
        self.fn = jax.jit(
            shard_map(_body, mesh=mesh, in_specs=in_specs, out_specs=out_specs,
                      check_rep=False),
            donate_argnums=donate_idx, keep_unused=True)
        self._jax = jax

    def device_args(self, in_maps):
        """Upload per-core inputs + zero output buffers once; reusable across calls."""
        import jax
        from jax.sharding import Mesh, NamedSharding, PartitionSpec
        per_core = [[np.asarray(m[n]) for n in self.in_names] for m in in_maps]
        concat_in = [np.concatenate([per_core[c][i] for c in range(self.n_cores)], axis=0)
                     for i in range(self.n_params)]
        concat_zeros = [np.zeros((self.n_cores * z.shape[0], *z.shape[1:]), z.dtype)
                        for z in self.zero_outs]
        mesh = Mesh(np.asarray(jax.devices()[:self.n_cores]), ("core",))
        sh = NamedSharding(mesh, PartitionSpec("core"))
        return [jax.device_put(a, sh) for a in concat_in + concat_zeros]

    def call_dev(self, dev_args):
        out_arrs = self.fn(*dev_args)
        self._jax.block_until_ready(out_arrs)
        return out_arrs

    def __call__(self, in_maps):
        out_arrs = self.call_dev(self.device_args(in_maps))
        return [
            {n: np.asarray(out_arrs[i]).reshape(self.n_cores, *self.out_avals[i].shape)[c]
             for i, n in enumerate(self.out_names)}
            for c in range(self.n_cores)
        ]


def _get_runner(steps):
    if steps not in _RUNNER_CACHE:
        nc = _build(steps)
        _RUNNER_CACHE[steps] = _Runner(nc)
    return _RUNNER_CACHE[steps]


def kernel(hx, cx, W, U, L, steps):
    steps = int(steps)
    ins = _prep_inputs(hx, cx, W, U, L)
    runner = _get_runner(steps)
    res = runner(ins)
    out = np.concatenate([res[k]["out"] for k in range(NC)], axis=2)  # [steps, 64, 512]
    return out.astype(np.float32)
